# revision 1
# baseline (speedup 1.0000x reference)
"""Trainium2 Bass kernel for nn_KMeansPalettizedLinear.

Computes y = x @ (lut[weight_idx])^T + bias for
  x: [4, 2048, 4096] f32, lut: [256] f32, weight_idx: [4096, 4096] i32,
  bias: [4096] f32  ->  y: [4, 2048, 4096] f32.

Strategy (column/tensor-parallel across 8 NeuronCores):
  - Host: dequantize W = lut[weight_idx] (palette gather), transpose X to
    X^T [D_IN, M], shard W^T/bias along out_features (512 per core).
  - Device (per core): Y_shard[m, o] = sum_d X^T[d, m] * W^T[d, o] + bias[o]
    as a tiled PE matmul with the X^T tile as the stationary operand
    (lhsT [128d, 128m]) and the SBUF-resident W^T as the moving operand
    ([128d, 512o]), accumulating over the 32 k-tiles in PSUM.
  - Matmul dtype is fp16 by default (PE upconverts to FP22 internally;
    ~1e-4 relative error) at full 1 cycle/row throughput.
"""

import os
import sys

sys.path.insert(0, "/opt/trn_rl_repo")

import numpy as np

B, S, D_IN, D_OUT, PALETTE = 4, 2048, 4096, 4096, 256
N_CORES = 8
M = B * S  # 8192
O_SHARD = D_OUT // N_CORES  # 512
P = 128
KO = D_IN // P  # 32 k-tiles
MG = M // 512  # 16 m-groups of 512 rows

# fp16 | bf16 | fp32r  (matmul input dtype; see module docstring)
MM_DTYPE = os.environ.get("KMEANS_MM_DTYPE", "fp16")
# >1 wraps the body in a device-side repeat loop (timing aid only)
REPEATS = int(os.environ.get("KMEANS_REPEATS", "1"))
# per-ko W-load split — cost model says the 32 small DMAs cost more than
# the prologue overlap saves (473us vs 464us), so default off
W_SPLIT = os.environ.get("KMEANS_W_SPLIT", "0") == "1"
X_BUFS = int(os.environ.get("KMEANS_X_BUFS", "12"))

_cache = {}


def _mm_dt():
    import concourse.mybir as mybir

    return {
        "fp16": (mybir.dt.float16, np.float16),
        "bf16": (mybir.dt.bfloat16, None),  # np side handled via ml_dtypes
        "fp32r": (mybir.dt.float32r, np.float32),
    }[MM_DTYPE]


def _np_cast(a):
    if MM_DTYPE == "fp16":
        return a.astype(np.float16)
    if MM_DTYPE == "bf16":
        import ml_dtypes

        return a.astype(ml_dtypes.bfloat16)
    return np.ascontiguousarray(a, dtype=np.float32)


def _build():
    from concourse import bacc
    import concourse.mybir as mybir
    import concourse.tile as tile
    from concourse.bass import ds, ts

    dt_mm, _ = _mm_dt()
    nc = bacc.Bacc(None, target_bir_lowering=False)
    xt = nc.dram_tensor("xt", [D_IN, M], dt_mm, kind="ExternalInput")
    wt = nc.dram_tensor("wt", [D_IN, O_SHARD], dt_mm, kind="ExternalInput")
    biasb = nc.dram_tensor("biasb", [P, O_SHARD], mybir.dt.float32, kind="ExternalInput")
    y = nc.dram_tensor("y", [M, O_SHARD], mybir.dt.float32, kind="ExternalOutput")

    with tile.TileContext(nc) as tc:
        with (
            tc.tile_pool(name="wpool", bufs=1) as wpool,
            tc.tile_pool(name="xpool", bufs=X_BUFS) as xpool,
            tc.tile_pool(name="opool", bufs=8) as opool,
            tc.tile_pool(name="cpool", bufs=1) as cpool,
            tc.tile_pool(name="psum", bufs=8, space="PSUM") as pp,
        ):
            w_res = wpool.tile([P, KO, O_SHARD], dt_mm)
            wt_r = wt.rearrange("(ko p) o -> p ko o", p=P)
            if W_SPLIT:
                # per-ko loads let the first matmuls start after 1/32 of W
                for ko in range(KO):
                    nc.sync.dma_start(w_res[:, ko, :], wt_r[:, ko, :])
            else:
                nc.sync.dma_start(w_res[:], wt_r)
            bias_t = cpool.tile([P, O_SHARD], mybir.dt.float32)
            nc.sync.dma_start(bias_t[:], biasb[:])

            import contextlib

            rep_ctx = (
                tc.For_i(0, REPEATS, 1) if REPEATS > 1 else contextlib.nullcontext()
            )
            with rep_ctx:
                _emit_body(nc, tc, xpool, opool, pp, w_res, bias_t, xt, y)
    nc.compile()
    return nc


def _emit_body(nc, tc, xpool, opool, pp, w_res, bias_t, xt, y):
    import concourse.mybir as mybir
    from concourse.bass import ds, ts

    dt_mm, _ = _mm_dt()
    if True:
            for mg in range(MG):
                psums = [
                    pp.tile([P, O_SHARD], mybir.dt.float32, tag="ps", name=f"ps_{mg}_{i}")
                    for i in range(4)
                ]
                for ko in range(KO):
                    xt_t = xpool.tile([P, 512], dt_mm, tag="xt")
                    nc.sync.dma_start(
                        xt_t[:], xt[ds(ko * P, P), ds(mg * 512, 512)]
                    )
                    for mi in range(4):
                        nc.tensor.matmul(
                            psums[mi][:],
                            xt_t[:, ts(mi, P)],
                            w_res[:, ko, :],
                            start=(ko == 0),
                            stop=(ko == KO - 1),
                        )
                for mi in range(4):
                    ot = opool.tile([P, O_SHARD], mybir.dt.float32, tag="ot")
                    nc.vector.tensor_tensor(
                        ot[:], psums[mi][:], bias_t[:], mybir.AluOpType.add
                    )
                    nc.sync.dma_start(y[ds(mg * 512 + mi * P, P), :], ot[:])


def get_nc():
    if "nc" not in _cache:
        _cache["nc"] = _build()
    return _cache["nc"]


def make_in_maps(input, lookup_table, weight_idx, bias):
    """Host-side shard/layout prep -> per-core input maps."""
    x = np.asarray(input, dtype=np.float32).reshape(M, D_IN)
    lut = np.asarray(lookup_table, dtype=np.float32)
    idx = np.asarray(weight_idx)
    b = np.asarray(bias, dtype=np.float32)

    xt = np.ascontiguousarray(_np_cast(x).T)  # [D_IN, M]
    wt_full = lut[idx].T  # [D_IN, D_OUT] f32 (palette dequant on host)

    in_maps = []
    for c in range(N_CORES):
        sl = slice(c * O_SHARD, (c + 1) * O_SHARD)
        in_maps.append(
            {
                "xt": xt,
                "wt": np.ascontiguousarray(_np_cast(wt_full[:, sl])),
                "biasb": np.ascontiguousarray(
                    np.broadcast_to(b[sl], (P, O_SHARD)), dtype=np.float32
                ),
            }
        )
    return in_maps


def kernel(input, lookup_table, weight_idx, bias):
    from concourse.bass_utils import run_bass_kernel_spmd

    nc = get_nc()
    in_maps = make_in_maps(input, lookup_table, weight_idx, bias)
    res = run_bass_kernel_spmd(nc, in_maps, core_ids=list(range(N_CORES)))
    y = np.concatenate([res.results[c]["y"] for c in range(N_CORES)], axis=1)
    return y.reshape(B, S, D_OUT)

